# revision 1
# baseline (speedup 1.0000x reference)
"""Trainium2 Bass kernel for nn_LocationSlayerRandom (SLAYER two-branch spiking net).

Contract: kernel(**inputs) takes the FULL unsharded inputs
  spike_input [32,156,1,1,2048] f32, W1 [512,156], W2 [20,512],
  Wl1 [512,2048], Wl2 [20,512], perm [156] i32
and returns the FULL output [32,20,1,1,2204] f32.

Strategy (8 cores, data-parallel over batch, 4 samples/core):

Branch 1 (per sample b):
  u1 = psp_t(W1 @ si) = W1 @ psp_t(si)            (psp is linear => commutes)
  - psp_t(si): DVE tensor_tensor_scan along t on the 156-row input
    (channels 0:127 per-b slices of one packed tile; channels 128:155 of all
    4 b packed into one 128-partition tile at offsets 32b, with per-sample
    zero-masked 128-row weight tiles selecting each sample's rows).
  - fc1 on PE in fp8 DoubleRow (contraction 156 as two k-subtiles: the
    packed-tail region with per-sample zero-masked weight rows, plus the
    128-channel region; one [128,2,512] rhs AP pairs regions {0, 1+b} of
    the psAB tile via a step slice, halving fc1 PE time vs bf16).
    Threshold fused straight from PSUM into fp8 encodings of s1 - 0.5:
    hidden m-tiles 0..2 on ACT as Sign(u1-10) in {-1,0,1} with fc2 weights
    pre-scaled 0.5; m-tile 3 on DVE as (u1>=10)-0.5 in {-.5,.5} with
    unscaled weights (engine load balancing). The affine 0.5*rowsum(W2)
    correction is folded into a host-side time-varying threshold
    T2[o,t] = 10 - 0.5*rowsum(W2_eff)[o]*g[t], g[t] = sum_{k<=t} alpha^k.
  - fc2 on PE in fp8, with the four samples packed into the four PE column
    groups (tile_position=(0,32b)) accumulating into ONE [128,2048] PSUM
    tile; one psp scan straight from PSUM; spike_output = (v >= T2).

Branch 2: ul1 = psp_c'(Wl1 @ x_tp) where x_tp[b,t,c'] = si[b,perm[c'],t].
  Host supplies the gathered+transposed input tiles sipT (pure layout prep),
  so the t-contraction runs with Wl1^T stationary and the c'-psp becomes a
  free-dim scan straight from PSUM with a reset-pattern data0 (alpha, but 0
  at each sample boundary). Then threshold, fc2, scan, threshold.

Numerics: matmuls fp8 (fc2-b2 bf16) with fp32 accumulate; raw spikes are
exact in fp8, psp scans keep f32 carry state and round only the stored
output elements. The only nonlinearity is the >=10 threshold; true layer-2
potentials sit below 3.2 (branch 1) / 2.0 (branch 2) against a threshold of
10 (host-verified min margin 7.0 with all fp8 rounding applied), so
near-threshold layer-1 bit flips from low-precision inputs/weights cannot
flip any output bit.
"""

from contextlib import ExitStack

import numpy as np
import ml_dtypes

import concourse.bass as bass
import concourse.mybir as mybir
from concourse import bacc
from concourse import tile as tile_mod
from concourse.bass_utils import run_bass_kernel_spmd

F32 = mybir.dt.float32
BF16 = mybir.dt.bfloat16
FP8 = mybir.dt.float8e4
DR = mybir.MatmulPerfMode.DoubleRow
AL = mybir.AluOpType
AF = mybir.ActivationFunctionType
BF16_NP = ml_dtypes.bfloat16
FP8_NP = ml_dtypes.float8_e4m3

B, C_IN, T = 32, 156, 2048
HID, OUT_DIM = 512, 20
CP = 156                      # permuted taxel axis (branch-2 "time")
N_CORES = 8
B_PER = B // N_CORES          # 4 samples per core
ALPHA = float(np.exp(-1.0 / 10.0))
THETA = 10.0
NB2 = B_PER * CP              # 624, branch-2 packed free dim
KT = T // 128                 # 16 k-tiles over t


def build_program(tc, outs, ins):
    nc = tc.nc
    out = outs["out"]

    with ExitStack() as ctx:
        consts = ctx.enter_context(tc.tile_pool(name="consts", bufs=1))
        work = ctx.enter_context(tc.tile_pool(name="work", bufs=1))
        sgp = ctx.enter_context(tc.tile_pool(name="sgp", bufs=16))
        mid = ctx.enter_context(tc.tile_pool(name="mid", bufs=4))
        psum1 = ctx.enter_context(tc.tile_pool(name="psum1", bufs=4, space="PSUM"))

        # ---------------- constant patterns (gpsimd; SBUF only) ----------
        alpha_t = consts.tile([128, T], F32, tag="alpha")
        nc.gpsimd.memset(alpha_t[:], ALPHA)
        pat624 = consts.tile([128, NB2], F32, tag="pat624")
        nc.gpsimd.memset(pat624[:], ALPHA)
        for j in range(B_PER):
            nc.gpsimd.memset(pat624[:, j * CP:j * CP + 1], 0.0)
        bias_m10 = consts.tile([128, 1], F32, tag="bm10")
        nc.gpsimd.memset(bias_m10[:], -THETA)
        act_warm = consts.tile([128, 1], F32, tag="actwarm")
        nc.scalar.activation(act_warm[:], bias_m10[:], AF.Sign,
                             bias=bias_m10[:])

        # ---------------- inputs (consolidated DMAs) ---------------------
        # branch-1 critical path first. siAB regions: 0 = packed tails
        # (rows 32b..32b+27), 1+b = sample b channels 0:128. fp8 (exact 0/1).
        siAB = consts.tile([128, 5 * T], FP8, tag="siAB")
        nc.sync.dma_start(siAB[:, T:2 * T], ins["siAB"][:, T:2 * T])
        nc.sync.dma_start(siAB[:, 0:T], ins["siAB"][:, 0:T])
        # branch-2 fp8 inputs early: A1(m0) runs during the scan ramp
        wl1 = consts.tile([128, KT * HID], FP8, tag="wl1")
        nc.sync.dma_start(wl1[:], ins["Wl1T"][:])
        sip = consts.tile([128, KT * NB2], FP8, tag="sip")
        nc.sync.dma_start(sip[:], ins["sipT"][:])
        for b in range(1, B_PER):        # per-sample slices so scan b starts
            nc.sync.dma_start(siAB[:, (1 + b) * T:(2 + b) * T],
                              ins["siAB"][:, (1 + b) * T:(2 + b) * T])
        # W1c regions: b = per-sample tail weights (rows 32b, zero-masked),
        # 4 = shared W1^T[0:128]; fp8 for DoubleRow fc1
        w1c = consts.tile([128, 5 * 512], FP8, tag="w1c")
        nc.sync.dma_start(w1c[:], ins["W1c"][:])
        w2p = consts.tile([128, 2 * 4 * 2 * 128], FP8, tag="w2p")
        nc.sync.dma_start(w2p[:], ins["W2pT"][:])
        t2_t = consts.tile([128, T], BF16, tag="t2")
        nc.sync.dma_start(t2_t[:], ins["T2"][:])
        wl2 = consts.tile([128, 4 * OUT_DIM], BF16, tag="wl2")
        nc.sync.dma_start(wl2[:], ins["Wl2T"][:])

        # ---------------- branch-1 input psp scans (DVE) -----------------
        # order: the packed tail region first, then sample 0 (fc1 b0 needs
        # both before its first accumulation group completes), then 1..3.
        psAB = work.tile([128, 5 * T], FP8, tag="psAB")
        psAB3 = psAB[:].rearrange("p (r t) -> p r t", t=T)
        w1c3 = w1c[:].rearrange("p (r mj) -> p r mj", mj=512)
        nc.vector.tensor_tensor_scan(psAB[:, T:2 * T], alpha_t[:],
                                     siAB[:, T:2 * T], 0.0, AL.mult, AL.add)
        nc.vector.tensor_tensor_scan(psAB[:, 0:T], alpha_t[:], siAB[:, 0:T],
                                     0.0, AL.mult, AL.add)

        # ---------------- branch 2 A1 block emitter (interleaved below) --
        # fp8 DoubleRow: two 128-row k-subtiles per pass ([128, 2, X] APs),
        # halving the pass count of the t-contraction. Emitted one m-block
        # after each fc1 sample so the PE fills fc1's threshold-paced gaps.
        wl1_3d = wl1[:].rearrange("p (k o) -> p k o", o=HID)
        sip_3d = sip[:].rearrange("p (k c) -> p k c", c=NB2)
        DR = mybir.MatmulPerfMode.DoubleRow
        l1 = []

        def a1_block(m):
            pa = psum1.tile([128, 1024], F32, tag="psum1", name=f"pa{m}")
            a1 = pa[:, :NB2]
            msl = slice(m * 128, (m + 1) * 128)
            for ki in range(KT // 2):
                st, sp = (ki == 0), (ki == KT // 2 - 1)
                lhs = wl1_3d[:, 2 * ki:2 * ki + 2, msl]
                nc.tensor.matmul(a1[:, 0:512], lhs,
                                 sip_3d[:, 2 * ki:2 * ki + 2, 0:512],
                                 start=st, stop=sp, perf_mode=DR)
                nc.tensor.matmul(a1[:, 512:NB2], lhs,
                                 sip_3d[:, 2 * ki:2 * ki + 2, 512:NB2],
                                 start=st, stop=sp, perf_mode=DR)
            u = mid.tile([128, NB2], BF16, tag="ul1", name=f"ul1{m}")
            nc.vector.tensor_tensor_scan(u[:], pat624[:], a1, 0.0,
                                         AL.mult, AL.add)
            lt = mid.tile([128, NB2], BF16, tag="l1", name=f"l1{m}")
            nc.vector.tensor_scalar(lt[:], u[:], THETA, None, AL.is_ge)
            l1.append(lt)

        # A1(m0) fills the PE-idle scan ramp; its scan slots into the DVE
        # stream before the remaining si scans.
        a1_block(0)
        for b in range(1, B_PER):
            nc.vector.tensor_tensor_scan(psAB[:, (1 + b) * T:(2 + b) * T],
                                         alpha_t[:],
                                         siAB[:, (1 + b) * T:(2 + b) * T],
                                         0.0, AL.mult, AL.add)

        # ---------------- branch 1 fc1 + fused Sign thresholds -----------
        # loop order b -> half -> m: the whole first inner phase consumes only
        # the first-half scans, so fc1 never stalls on a later half-scan.
        # sg pair tiles per (b, kp): [128, 2*T] fp8, layout [p, (s t)] so
        # fc2 can pair the two m-tiles of kp as DoubleRow k-subtiles.
        sgt = {}
        for b in range(B_PER):
            if b >= 2:
                a1_block(b - 1)
            for kp in range(2):
                sgt[(b, kp)] = sgp.tile([128, 2 * T], FP8, tag="sg",
                                        name=f"sg{b}{kp}")
            for m in range(4):
                for half in range(2):
                    s_t = sgt[(b, m // 2)][:, (m % 2) * T:(m % 2) * T + T]
                    msl = slice(m * 128, (m + 1) * 128)
                    # fp8 DoubleRow: sub-0 = (tail weights b, packed-tail
                    # psp), sub-1 = (main weights, sample-b psp)
                    lhsT = w1c3[:, b:5:(4 - b), msl]
                    pu = psum1.tile([128, 1024], F32, tag="psum1")
                    for ch in range(2):
                        c0 = half * 1024 + ch * 512
                        nc.tensor.matmul(pu[:, ch * 512:(ch + 1) * 512],
                                         lhsT,
                                         psAB3[:, 0:b + 2:b + 1, c0:c0 + 512],
                                         start=True, stop=True, perf_mode=DR)
                    hsl = slice(half * 1024, (half + 1) * 1024)
                    if m < 2:
                        # ACT: sg = sign(u1-10) in {-1,0,1}; weights 0.5x
                        nc.scalar.activation(s_t[:, hsl], pu[:], AF.Sign,
                                             bias=bias_m10[:])
                    else:
                        # DVE: sg = (u1>=10)-0.5 in {-.5,.5}; weights 1.0x
                        nc.vector.tensor_scalar(s_t[:, hsl], pu[:], THETA, 0.5,
                                                AL.is_ge, AL.subtract)

        a1_block(3)

        # branch 2 fc2 + psp + threshold + out (emitted BEFORE branch-1 fc2:
        # its l1 inputs are ready during fc1 thanks to the A1 interleave, so
        # its whole chain hides under the fc2-b1 matmuls below)
        pl2full = psum1.tile([128, 1024], F32, tag="psum1")
        pl2 = pl2full[:OUT_DIM, :NB2]
        for k in range(4):
            st, sp = (k == 0), (k == 3)
            ksl = slice(k * OUT_DIM, (k + 1) * OUT_DIM)
            nc.tensor.matmul(pl2[:, 0:512], wl2[:, ksl], l1[k][:, 0:512],
                             start=st, stop=sp)
            nc.tensor.matmul(pl2[:, 512:NB2], wl2[:, ksl], l1[k][:, 512:NB2],
                             start=st, stop=sp)
        ul2 = mid.tile([128, NB2], F32, tag="ul2")
        nc.vector.tensor_tensor_scan(ul2[:OUT_DIM], pat624[:OUT_DIM], pl2, 0.0,
                                     AL.mult, AL.add)
        o2 = mid.tile([128, NB2], BF16, tag="o2")
        nc.vector.tensor_scalar(o2[:OUT_DIM], ul2[:OUT_DIM], THETA, None,
                                AL.is_ge)
        nc.sync.dma_start(
            out[:, :OUT_DIM, T:T + CP].rearrange("b o c -> o b c"),
            o2[:OUT_DIM, :].rearrange("o (b c) -> o b c", c=CP))

        # ---------------- branch 1 fc2, col-tiled over samples -----------
        # t-half at a time: the first half's scan + compare + output DMA
        # overlap the second half's matmuls, shrinking the kernel tail.
        vs = work.tile([128, T], BF16, tag="vs")
        o1 = work.tile([128, T], BF16, tag="o1")
        vcarry = work.tile([128, 1], F32, tag="vcarry")
        vmid = work.tile([128, 1], F32, tag="vmid")
        prev_init = 0.0
        w2blk4 = w2p[:].rearrange("p (kp b s j) -> p kp b s j",
                                  b=4, s=2, j=128)
        for hf in range(2):
            puh = psum1.tile([128, 1024], F32, tag="psum1", name=f"pu2{hf}")
            for ch in range(2):
                c0 = hf * 1024 + ch * 512
                for kp in range(2):
                    for b in range(B_PER):
                        # DoubleRow over the m-pair of kp; per-(kp,b) block
                        # weights place sample b's 20 outputs at rows 32b
                        # (zero elsewhere), so all 8 matmuls accumulate one
                        # full-width [128,512] group (start once).
                        rhs3 = sgt[(b, kp)][:].rearrange(
                            "p (s t) -> p s t", t=T)
                        nc.tensor.matmul(puh[:, ch * 512:(ch + 1) * 512],
                                         w2blk4[:, kp, b, :, :],
                                         rhs3[:, :, c0:c0 + 512],
                                         start=(kp == 0 and b == 0),
                                         stop=(kp == 1 and b == 3),
                                         perf_mode=DR)
            # split v-scan at the PSUM ch-group boundary: the first 512
            # cols scan while the second ch-group's matmuls still run
            q0 = hf * 1024
            nc.vector.tensor_tensor_scan(vs[:, q0:q0 + 512],
                                         alpha_t[:, 0:512], puh[:, 0:512],
                                         prev_init, AL.mult, AL.add)
            nc.vector.tensor_scalar(vmid[:], vs[:, q0 + 511:q0 + 512],
                                    1.0, None, AL.mult)
            nc.vector.tensor_tensor_scan(vs[:, q0 + 512:q0 + 1024],
                                         alpha_t[:, 0:512],
                                         puh[:, 512:1024],
                                         vmid[:, 0:1], AL.mult, AL.add)
            if hf == 0:
                nc.vector.tensor_scalar(vcarry[:], vs[:, 1023:1024],
                                        1.0, None, AL.mult)
                prev_init = vcarry[:, 0:1]
            for q in range(2):
                qsl = slice(q0 + q * 512, q0 + (q + 1) * 512)
                nc.vector.tensor_tensor(o1[:, qsl], vs[:, qsl], t2_t[:, qsl],
                                        AL.is_ge)
                nc.sync.dma_start(
                    out[:, :, qsl].rearrange("b j t -> (b j) t"), o1[:, qsl])


# ======================= host-side preparation =======================

def prep_core_inputs(si, sip, core):
    """Per-core data tensors, pre-packed into single-DMA SBUF layouts.
    si/sip are [32,156,2048] f32 (sip already perm-gathered)."""
    sl = si[core * B_PER:(core + 1) * B_PER]          # [4,156,2048]
    # siAB [128, 5*T] fp8: region 0 = packed tails (rows 32b..32b+27),
    # region 1+b = si[b, 0:128, :]
    siAB = np.zeros((128, 5 * T), dtype=FP8_NP)
    for b in range(B_PER):
        siAB[:, (1 + b) * T:(2 + b) * T] = sl[b, :128, :]
        siAB[32 * b:32 * b + (C_IN - 128), 0:T] = sl[b, 128:C_IN, :]
    sp = sip[core * B_PER:(core + 1) * B_PER]         # [4,156,2048]
    # sipT [128, KT*NB2]: [p, k*NB2 + b*CP + c'] = sip[b, c', 128k+p]
    sipT = np.ascontiguousarray(
        sp.transpose(2, 0, 1).reshape(KT, 128, NB2)
        .transpose(1, 0, 2).reshape(128, KT * NB2)
    ).astype(FP8_NP)
    return {"siAB": siAB, "sipT": sipT}


def prep_shared_inputs(W1, W2, Wl1, Wl2):
    """Weight layouts + threshold tensor, shared by all cores."""
    # W1c [128, 5*512] fp8: region b = tail weights for sample b at rows
    # 32b..32b+27 (zero elsewhere); region 4 = W1^T[0:128].
    w1t = np.zeros((160, HID), dtype=np.float32)
    w1t[:C_IN] = W1.T
    W1c = np.zeros((128, 5 * 512), dtype=FP8_NP)
    W1c[:, 4 * 512:5 * 512] = w1t[:128]
    for b in range(B_PER):
        W1c[32 * b:32 * b + (C_IN - 128), b * 512:(b + 1) * 512] = \
            w1t[128:C_IN]

    # fc2 block weights for DoubleRow: per (kp, b, s) a [128, 128] block,
    # zero except cols 32b..32b+20 = k_scale[m]*W2_m^T (m = 2kp+s). Per-m
    # scale matches the sg encoding: ACT Sign (+-1) -> 0.5x, DVE (+-.5)
    # -> 1.0x. Layout [128, 2*4*2*128]: [p, ((kp*4+b)*2+s)*128 + j]
    k_scale = (0.5, 0.5, 1.0, 1.0)
    w2t = W2.T.astype(np.float32)                     # [512, 20]
    W2pT = np.zeros((128, 2 * 4 * 2 * 128), dtype=FP8_NP)
    for kp in range(2):
        for s in range(2):
            m = 2 * kp + s
            blk = (k_scale[m] * w2t[m * 128:(m + 1) * 128]).astype(FP8_NP)
            for b in range(B_PER):
                base = ((kp * 4 + b) * 2 + s) * 128 + 32 * b
                W2pT[:, base:base + OUT_DIM] = blk
    # effective (device) W2 after fp8 rounding, unscaled
    w2_eff = np.empty((HID, OUT_DIM), dtype=np.float32)
    for m in range(4):
        kp, s = m // 2, m % 2
        base = ((kp * 4 + 0) * 2 + s) * 128 + 0
        w2_eff[m * 128:(m + 1) * 128] = (
            W2pT[:, base:base + OUT_DIM].astype(np.float32) / k_scale[m])
    r2 = w2_eff.sum(axis=0)                           # [20]
    g = (1.0 - ALPHA ** (np.arange(T, dtype=np.float64) + 1)) / (1.0 - ALPHA)
    theta2 = (THETA - 0.5 * np.outer(r2, g)).astype(np.float32)   # [20, T]
    T2 = np.full((128, T), 1e30, dtype=np.float32)
    for b in range(B_PER):
        T2[32 * b:32 * b + OUT_DIM] = theta2
    T2 = T2.astype(BF16_NP)

    # Wl1T [128, KT*HID]: [p, k*HID+o] = Wl1[o, 128k+p]
    Wl1T = np.ascontiguousarray(
        Wl1.T.reshape(KT, 128, HID).transpose(1, 0, 2).reshape(128, KT * HID)
    ).astype(FP8_NP)
    # Wl2T [128, 4*OUT]: [p, k*OUT+o] = Wl2[o, 128k+p]
    Wl2T = np.ascontiguousarray(
        Wl2.T.reshape(4, 128, OUT_DIM).transpose(1, 0, 2).reshape(128, 4 * OUT_DIM)
    ).astype(BF16_NP)
    return {"W1c": W1c, "W2pT": W2pT, "Wl1T": Wl1T,
            "Wl2T": Wl2T, "T2": T2}


def make_in_maps(spike_input, W1, W2, Wl1, Wl2, perm):
    si = np.asarray(spike_input, dtype=np.float32).reshape(B, C_IN, T)
    perm = np.asarray(perm).astype(np.int64)
    sip = si[:, perm, :]                              # perm-gather (layout only)
    shared = prep_shared_inputs(np.asarray(W1, np.float32),
                                np.asarray(W2, np.float32),
                                np.asarray(Wl1, np.float32),
                                np.asarray(Wl2, np.float32))
    in_maps = []
    for core in range(N_CORES):
        m = dict(shared)
        m.update(prep_core_inputs(si, sip, core))
        in_maps.append(m)
    return in_maps


_IN_SPECS = {
    "siAB": ((128, 5 * T), FP8),
    "sipT": ((128, KT * NB2), FP8),
    "W1c": ((128, 5 * 512), FP8),
    "W2pT": ((128, 2 * 4 * 2 * 128), FP8),
    "Wl1T": ((128, KT * HID), FP8),
    "Wl2T": ((128, 4 * OUT_DIM), BF16),
    "T2": ((128, T), BF16),
}


def build_bass():
    nc = bacc.Bacc("TRN2", target_bir_lowering=False, debug=False)
    ins = {}
    for name, (shape, dt) in _IN_SPECS.items():
        h = nc.dram_tensor(name, list(shape), dt, kind="ExternalInput")
        ins[name] = h[:]
    out_h = nc.dram_tensor("out", [B_PER, 32, T + CP], BF16,
                           kind="ExternalOutput")
    outs = {"out": out_h[:]}
    with tile_mod.TileContext(nc) as tc:
        build_program(tc, outs, ins)
    nc.compile()
    return nc


_NC_CACHE = None


def run(inputs, trace=False, **kw):
    """Run on the 8 NeuronCores; returns (full_output, BassKernelResults)."""
    global _NC_CACHE
    if _NC_CACHE is None:
        _NC_CACHE = build_bass()
    nc = _NC_CACHE
    in_maps = make_in_maps(**inputs)
    res = run_bass_kernel_spmd(nc, in_maps, core_ids=list(range(N_CORES)),
                               trace=trace, **kw)
    parts = [res.results[c]["out"][:, :OUT_DIM, :] for c in range(N_CORES)]
    full = np.concatenate(parts, axis=0).reshape(B, OUT_DIM, 1, 1, T + CP)
    return np.ascontiguousarray(full.astype(np.float32)), res


def kernel(**inputs):
    out, _ = run(inputs)
    return out

